# revision 7
# baseline (speedup 1.0000x reference)
"""Trainium2 Bass kernel for a teacher-forced LSTM decoder.

reference math:
    h0 = mean(enc_h, axis=0); c0 = mean(enc_c, axis=0)
    tok = [SOS, y[:, :-1]]
    gx[b,t] = relu(emb[tok[b,t]]) @ W_ih.T + b_ih + b_hh     # (B,T,4H)
    scan over t:  gates = gx_t + h @ W_hh.T ; LSTM cell (torch gate order i,f,g,o)
    logits = hs @ W_out.T + b_out ; log_probs = log_softmax(logits)
    returns (log_probs, h_T[None], c_T[None])

Strategy (8 NeuronCores, data-parallel over batch B=128 -> 16 rows/core):
  - G table: gx rows only depend on the token id, so precompute
    G = relu(emb) @ W_ih.T + (b_ih + b_hh)  (V x 4H, bf16) once per core and
    gather 16 rows per step with an indirect DMA.
  - recurrence: per step, gates PSUM accumulates [identity-matmul of the
    gathered G rows] + [4 k-chunks of h @ W_hh.T in bf16]. Gate chunks are
    host-permuted to (f, i, g, o) order so each PSUM bank is one gate.
    h is transposed back to (H, B) layout on the PE each step into a
    persistent hsT history buffer, which is also the stationary operand for
    the output projection.
  - projection: 128-row (t,b) m-tiles vs W_out.T (+bias via ones-row matmul),
    Exp with fused row-sum (accum_out), Ln batched per 8 tiles (single ACT
    table set switch per group), subtract, DMA out.
"""

import numpy as np
import ml_dtypes

import concourse.bass as bass
import concourse.mybir as mybir
import concourse.tile as tile
from concourse import bacc
from concourse.bass import IndirectOffsetOnAxis
from concourse.bass_utils import run_bass_kernel_spmd
from concourse.masks import make_identity

B, T, V, E, H = 128, 512, 1024, 256, 512
G4 = 4 * H
N_CORES = 8
BL = B // N_CORES  # 16 batch rows per core

F32 = mybir.dt.float32
BF16 = mybir.dt.bfloat16
I32 = mybir.dt.int32
AF = mybir.ActivationFunctionType

KC_H = H // 128  # 4 k-chunks over H
KC_E = E // 128  # 2 k-chunks over E
NB = G4 // 512   # 4 gate chunks (f, i, g, o after host permute)
TPM = 128 // BL  # 8 time steps per 128-row projection m-tile


def build_bass(t_steps: int = T):
    nc = bacc.Bacc(trn_type="TRN2")

    embT = nc.dram_tensor("embT", [E, V], BF16, kind="ExternalInput")
    w_ihT = nc.dram_tensor("w_ihT", [E, G4], BF16, kind="ExternalInput")
    w_hhT = nc.dram_tensor("w_hhT", [H, G4], BF16, kind="ExternalInput")
    biasr = nc.dram_tensor("biasr", [1, G4], BF16, kind="ExternalInput")
    w_outT = nc.dram_tensor("w_outT", [H, V], BF16, kind="ExternalInput")
    boutr = nc.dram_tensor("boutr", [1, V], BF16, kind="ExternalInput")
    tok = nc.dram_tensor("tok", [BL, t_steps], I32, kind="ExternalInput")
    enc_h = nc.dram_tensor("enc_h", [2, BL, H], F32, kind="ExternalInput")
    enc_c = nc.dram_tensor("enc_c", [2, BL, H], F32, kind="ExternalInput")

    lp_out = nc.dram_tensor("lp_out", [BL, t_steps, V], F32, kind="ExternalOutput")
    h_out = nc.dram_tensor("h_out", [BL, H], F32, kind="ExternalOutput")
    c_out = nc.dram_tensor("c_out", [BL, H], F32, kind="ExternalOutput")

    gtab = nc.dram_tensor("gtab", [V, G4], BF16, kind="Internal")

    with tile.TileContext(nc) as tc:
        with tc.tile_pool(name="const", bufs=1) as const:
            ident = const.tile([BL, BL], F32)
            make_identity(nc, ident)
            i16b = const.tile([BL, BL], BF16)
            nc.vector.tensor_copy(i16b, ident)
            ones128 = const.tile([1, 128], BF16)
            nc.vector.memset(ones128, 1.0)
            bias_sb = const.tile([1, G4], BF16)
            nc.sync.dma_start(out=bias_sb, in_=biasr[:, :])
            bout_sb = const.tile([1, V], BF16)
            nc.sync.dma_start(out=bout_sb, in_=boutr[:, :])
            idx_sb = const.tile([BL, t_steps], I32)
            nc.sync.dma_start(out=idx_sb, in_=tok[:, :])
            # transposed h history: hsT[:, k, t, b] = h_{t-1}[b, 128k + p]
            hsT = const.tile([128, KC_H, t_steps + 1, BL], BF16)

            # ---------------- Phase 0: G table ----------------
            with tc.tile_pool(name="gb_sb", bufs=2) as gb_sb, \
                 tc.tile_pool(name="gb_w", bufs=1) as gb_w, \
                 tc.tile_pool(name="gb_ps", bufs=2, space="PSUM") as gb_ps:
                embT_sb = gb_w.tile([128, KC_E, V], BF16)
                nc.sync.dma_start(
                    out=embT_sb, in_=embT.rearrange("(k p) v -> p k v", p=128))
                nc.scalar.activation(out=embT_sb, in_=embT_sb, func=AF.Relu)
                wih_sb = gb_w.tile([128, KC_E, G4], BF16)
                nc.sync.dma_start(
                    out=wih_sb, in_=w_ihT.rearrange("(k p) g -> p k g", p=128))
                for m in range(V // 128):
                    ps = gb_ps.tile([128, G4], F32)
                    for nb in range(NB):
                        nsl = bass.ts(nb, 512)
                        nc.tensor.matmul(ps[:, nsl], lhsT=ones128,
                                         rhs=bias_sb[:, nsl], start=True, stop=False)
                        for k in range(KC_E):
                            nc.tensor.matmul(
                                ps[:, nsl], lhsT=embT_sb[:, k, bass.ts(m, 128)],
                                rhs=wih_sb[:, k, nsl],
                                start=False, stop=(k == KC_E - 1))
                    gt = gb_sb.tile([128, G4], BF16)
                    nc.vector.tensor_copy(gt, ps)
                    nc.sync.dma_start(out=gtab[bass.ts(m, 128), :], in_=gt)

            # ---------------- Phase 1: recurrence ----------------
            with tc.tile_pool(name="whh", bufs=1) as whh_p, \
                 tc.tile_pool(name="gx", bufs=3) as gx_p, \
                 tc.tile_pool(name="ew", bufs=2) as ew_p, \
                 tc.tile_pool(name="hcs", bufs=3) as hcs_p, \
                 tc.tile_pool(name="gps", bufs=6, space="PSUM") as gps_p, \
                 tc.tile_pool(name="tps", bufs=2, space="PSUM") as tps_p:
                whh_sb = whh_p.tile([128, KC_H, G4], BF16)
                nc.sync.dma_start(
                    out=whh_sb, in_=w_hhT.rearrange("(k p) g -> p k g", p=128))

                def emit_hT(h_tile, slot):
                    tp = tps_p.tile([128, KC_H, BL], F32)
                    for k in range(KC_H):
                        nc.tensor.transpose(
                            out=tp[:, k, :], in_=h_tile[:, bass.ts(k, 128)],
                            identity=ident)
                    nc.vector.tensor_copy(hsT[:, :, slot, :], tp)

                # h0 = mean(enc_h, 0), c0 = mean(enc_c, 0)
                ehl = ew_p.tile([BL, 2, H], F32, tag="ld0")
                nc.sync.dma_start(out=ehl, in_=enc_h.rearrange("l b h -> b l h"))
                hsum = ew_p.tile([BL, H], F32, tag="tm")
                nc.vector.tensor_add(hsum, ehl[:, 0, :], ehl[:, 1, :])
                h_prev = hcs_p.tile([BL, H], F32, tag="h")
                nc.vector.tensor_scalar_mul(h_prev, hsum, 0.5)
                ecl = ew_p.tile([BL, 2, H], F32, tag="ld1")
                nc.sync.dma_start(out=ecl, in_=enc_c.rearrange("l b h -> b l h"))
                csum = ew_p.tile([BL, H], F32, tag="tm")
                nc.vector.tensor_add(csum, ecl[:, 0, :], ecl[:, 1, :])
                c_prev = hcs_p.tile([BL, H], F32, tag="c")
                nc.vector.tensor_scalar_mul(c_prev, csum, 0.5)
                emit_hT(h_prev, 0)

                h_last = h_prev
                for t in range(t_steps):
                    gx = gx_p.tile([BL, G4], BF16)
                    nc.gpsimd.indirect_dma_start(
                        out=gx, out_offset=None, in_=gtab[:, :],
                        in_offset=IndirectOffsetOnAxis(ap=idx_sb[:, t:t + 1], axis=0))
                    gp = []
                    for nb in range(NB):
                        g = gps_p.tile([BL, 512], F32, tag="g")
                        nc.tensor.matmul(g, lhsT=i16b, rhs=gx[:, bass.ts(nb, 512)],
                                         start=True, stop=False)
                        for k in range(KC_H):
                            nc.tensor.matmul(
                                g, lhsT=hsT[:, k, t, :],
                                rhs=whh_sb[:, k, bass.ts(nb, 512)],
                                start=False, stop=(k == KC_H - 1))
                        gp.append(g)
                    # gate chunk order after host permute: f, i, g, o
                    sf = ew_p.tile([BL, 512], F32, tag="sf")
                    nc.scalar.activation(sf, gp[0], AF.Sigmoid)
                    si = ew_p.tile([BL, 512], F32, tag="si")
                    nc.scalar.activation(si, gp[1], AF.Sigmoid)
                    tg = ew_p.tile([BL, 512], F32, tag="tg")
                    nc.scalar.activation(tg, gp[2], AF.Tanh)
                    so = ew_p.tile([BL, 512], F32, tag="so")
                    nc.scalar.activation(so, gp[3], AF.Sigmoid)
                    cn = hcs_p.tile([BL, H], F32, tag="c")
                    nc.vector.tensor_mul(cn, sf, c_prev)
                    tm = ew_p.tile([BL, H], F32, tag="tm")
                    nc.vector.tensor_mul(tm, si, tg)
                    nc.vector.tensor_add(cn, cn, tm)
                    tcl = ew_p.tile([BL, H], F32, tag="tc")
                    nc.scalar.activation(tcl, cn, AF.Tanh)
                    hn = hcs_p.tile([BL, H], F32, tag="h")
                    nc.vector.tensor_mul(hn, so, tcl)
                    emit_hT(hn, t + 1)
                    c_prev = cn
                    h_last = hn
                nc.sync.dma_start(out=h_out[:, :], in_=h_last)
                nc.sync.dma_start(out=c_out[:, :], in_=c_prev)

            # ---------------- Phase 2: projection + log_softmax ----------------
            with tc.tile_pool(name="wout", bufs=1) as wo_p, \
                 tc.tile_pool(name="park", bufs=10) as park_p, \
                 tc.tile_pool(name="sums", bufs=2) as sums_p, \
                 tc.tile_pool(name="lgps", bufs=2, space="PSUM") as lg_p:
                wout_sb = wo_p.tile([128, KC_H, V], BF16)
                nc.sync.dma_start(
                    out=wout_sb, in_=w_outT.rearrange("(k p) v -> p k v", p=128))
                n_mt = (t_steps * BL) // 128
                GRP = 8
                for g0 in range(0, n_mt, GRP):
                    ng = min(GRP, n_mt - g0)
                    sums = sums_p.tile([128, GRP], F32, tag="sums")
                    parks = []
                    for j in range(ng):
                        mt = g0 + j
                        t0 = mt * TPM
                        ps = lg_p.tile([128, V], F32)
                        for nb2 in range(V // 512):
                            nsl = bass.ts(nb2, 512)
                            nc.tensor.matmul(ps[:, nsl], lhsT=ones128,
                                             rhs=bout_sb[:, nsl],
                                             start=True, stop=False)
                            for k in range(KC_H):
                                lhsT = hsT[:, k, t0 + 1:t0 + 1 + TPM, :]
                                lhsT = lhsT.rearrange("p a b -> p (a b)")
                                nc.tensor.matmul(
                                    ps[:, nsl], lhsT=lhsT, rhs=wout_sb[:, k, nsl],
                                    start=False, stop=(k == KC_H - 1))
                        pk = park_p.tile([128, V], F32, tag="park")
                        nc.vector.tensor_copy(pk, ps)
                        ex = park_p.tile([128, V], F32, tag="expscratch")
                        nc.scalar.activation(out=ex, in_=ps, func=AF.Exp,
                                             accum_out=sums[:, j:j + 1])
                        parks.append((pk, t0))
                    lns = sums_p.tile([128, GRP], F32, tag="lns")
                    nc.scalar.activation(lns[:, 0:ng], sums[:, 0:ng], AF.Ln)
                    for j, (pk, t0) in enumerate(parks):
                        nc.vector.tensor_scalar_sub(pk, pk, lns[:, j:j + 1])
                        dst = bass.AP(
                            tensor=lp_out[:, :, :].tensor,
                            offset=t0 * V,
                            ap=[[V, TPM], [t_steps * V, BL], [1, V]],
                        )
                        nc.sync.dma_start(out=dst, in_=pk[:, :])
    nc.compile()
    return nc


_CACHE = {}


def _get_bass(t_steps):
    if t_steps not in _CACHE:
        _CACHE[t_steps] = build_bass(t_steps)
    return _CACHE[t_steps]


def run(inputs, t_steps=T, trace=False, tmpdir=None):
    bf = ml_dtypes.bfloat16
    enc_h = np.asarray(inputs["enc_h"], np.float32)
    enc_c = np.asarray(inputs["enc_c"], np.float32)
    tgt = np.asarray(inputs["target_tensor"])[:, :t_steps]
    emb = np.asarray(inputs["embedding"], np.float32)
    W_ih = np.asarray(inputs["W_ih"], np.float32)
    W_hh = np.asarray(inputs["W_hh"], np.float32)
    b_ih = np.asarray(inputs["b_ih"], np.float32)
    b_hh = np.asarray(inputs["b_hh"], np.float32)
    W_out = np.asarray(inputs["W_out"], np.float32)
    b_out = np.asarray(inputs["b_out"], np.float32)

    # permute torch gate order (i, f, g, o) -> chunk order (f, i, g, o)
    perm = np.concatenate([
        np.arange(H, 2 * H), np.arange(0, H),
        np.arange(2 * H, 3 * H), np.arange(3 * H, 4 * H)])
    embT = np.ascontiguousarray(emb.T).astype(bf)
    w_ihT = np.ascontiguousarray(W_ih[perm].T).astype(bf)
    w_hhT = np.ascontiguousarray(W_hh[perm].T).astype(bf)
    biasr = (b_ih + b_hh)[perm].astype(bf)[None, :]
    w_outT = np.ascontiguousarray(W_out.T).astype(bf)
    boutr = b_out.astype(bf)[None, :]
    toks = np.concatenate(
        [np.zeros((B, 1), tgt.dtype), tgt[:, :-1]], axis=1).astype(np.int32)

    in_maps = []
    for c in range(N_CORES):
        sl = slice(c * BL, (c + 1) * BL)
        in_maps.append({
            "embT": embT, "w_ihT": w_ihT, "w_hhT": w_hhT, "biasr": biasr,
            "w_outT": w_outT, "boutr": boutr,
            "tok": np.ascontiguousarray(toks[sl]),
            "enc_h": np.ascontiguousarray(enc_h[:, sl, :]),
            "enc_c": np.ascontiguousarray(enc_c[:, sl, :]),
        })

    nc = _get_bass(t_steps)
    res = run_bass_kernel_spmd(nc, in_maps, core_ids=list(range(N_CORES)),
                               trace=trace, tmpdir=tmpdir)
    lp = np.concatenate([r["lp_out"] for r in res.results], axis=0)
    hT = np.concatenate([r["h_out"] for r in res.results], axis=0)[None]
    cT = np.concatenate([r["c_out"] for r in res.results], axis=0)[None]
    return (lp, hT, cT), res


def kernel(encoder_outputs, enc_h, enc_c, target_tensor, embedding,
           W_ih, W_hh, b_ih, b_hh, W_out, b_out):
    (lp, hT, cT), _ = run(dict(
        enc_h=enc_h, enc_c=enc_c, target_tensor=target_tensor,
        embedding=embedding, W_ih=W_ih, W_hh=W_hh, b_ih=b_ih, b_hh=b_hh,
        W_out=W_out, b_out=b_out))
    return lp, hT, cT
